# revision 1
# baseline (speedup 1.0000x reference)
"""Trainium2 Bass kernel for nn_MultiHeadAttention (B=2, S=2048, E=1024, H=16, d=64).

Sharding: 8 cores = 2 batches x 4 head-groups (4 heads each).
Per core: QKV projection (transposed layout), causal flash-style attention
(transposed softmax, no max subtraction), o_proj partial; host sums the
4 partials per batch (the tensor-parallel all-reduce, done at gather time).

All matmuls in bf16 with fp32 PSUM accumulation.
"""
import sys

sys.path.insert(0, "/opt/trn_rl_repo")

import math

import ml_dtypes
import numpy as np

import concourse.bacc as bacc_mod
import concourse.tile as tile
from concourse import mybir
from concourse.bass_utils import run_bass_kernel_spmd

F32 = mybir.dt.float32
BF16 = mybir.dt.bfloat16
AF = mybir.ActivationFunctionType
ALU = mybir.AluOpType

B, S, E = 2, 2048, 1024
H, D = 16, 64           # total heads, head dim
HG = 4                  # heads per core (group)
NC_ = 8                 # cores
SC = S // 128           # 16 sequence chunks of 128
INV_SQRT_D = 1.0 / math.sqrt(D)

BF = ml_dtypes.bfloat16


def build_nc():
    nc = bacc_mod.Bacc(target_bir_lowering=False)

    # ---- DRAM tensors (per-core shards, prepared on host) ----
    xt_d = nc.dram_tensor("xt", [E, S], BF16, kind="ExternalInput")          # X[b].T
    wqk_d = nc.dram_tensor("wqk", [E, 2 * HG * D], BF16, kind="ExternalInput")
    wv_d = nc.dram_tensor("wv", [E, HG * D], BF16, kind="ExternalInput")
    wo_d = nc.dram_tensor("wo", [128, 2, E], BF16, kind="ExternalInput")
    ident_d = nc.dram_tensor("ident", [128, 128], BF16, kind="ExternalInput")
    tneg_d = nc.dram_tensor("tneg", [128, 128], BF16, kind="ExternalInput")
    bqk_d = nc.dram_tensor("bqk", [128, 4], F32, kind="ExternalInput")
    bv_d = nc.dram_tensor("bv", [128, HG * D], F32, kind="ExternalInput")
    y_d = nc.dram_tensor("y", [S, E], BF16, kind="ExternalOutput")

    with tile.TileContext(nc) as tc:
        import contextlib
        with contextlib.ExitStack() as ctx:
            persist = ctx.enter_context(tc.tile_pool(name="persist", bufs=1))
            dve_tmp = ctx.enter_context(tc.tile_pool(name="dve_tmp", bufs=4))
            expt_pool = ctx.enter_context(tc.tile_pool(name="expt_pool", bufs=7))

            # ---- persistent SBUF tiles ----
            xt = persist.tile([128, 8, S], BF16)            # X.T  [e-part, e-chunk, s]
            wqk = persist.tile([128, 8, 2 * HG * D], BF16)
            wv = persist.tile([128, 8, HG * D], BF16)
            wo = persist.tile([128, 2, E], BF16)
            ident = persist.tile([128, 128], BF16)
            tneg = persist.tile([128, 128], BF16)
            bqk = persist.tile([128, 4], F32)
            bv = persist.tile([128, HG * D], F32)
            qt = persist.tile([128, 2, S], BF16)            # Q.T
            kt = persist.tile([128, 2, S], BF16)            # K.T
            # V-hat: col 0 = ones (denominator row), cols 1-63 zero pad
            # (engine partition APs must fit aligned power-of-2 blocks),
            # cols 64-127 = V
            vh = persist.tile([128, SC, HG, 128], BF16)
            outt = persist.tile([128, 2, S], BF16)          # even-head staging (rows 64-127)
            outt2 = persist.tile([128, 2, S], BF16)         # stacked for o_proj

            xt_dr = xt_d.ap().rearrange("(o p) s -> p o s", p=128)
            wqk_dr = wqk_d.ap().rearrange("(o p) f -> p o f", p=128)
            for e in range(0, 8, 2):
                nc.sync.dma_start(wqk[:, e:e + 2, :], wqk_dr[:, e:e + 2, :])
                nc.sync.dma_start(xt[:, e:e + 2, 0:512], xt_dr[:, e:e + 2, 0:512])
            nc.sync.dma_start(wv[:], wv_d.ap().rearrange("(o p) f -> p o f", p=128))
            nc.sync.dma_start(ident[:], ident_d.ap())
            nc.sync.dma_start(tneg[:], tneg_d.ap())
            nc.sync.dma_start(bqk[:], bqk_d.ap())
            nc.sync.dma_start(bv[:], bv_d.ap())
            nc.sync.dma_start(wo[:], wo_d.ap())
            nc.vector.memset(vh[:], 0.0)
            nc.vector.memset(vh[:, :, :, 0], 1.0)

            ab_ctx = ctx.enter_context(contextlib.ExitStack())
            ps_a = ab_ctx.enter_context(tc.tile_pool(name="ps_a", bufs=2, space="PSUM"))
            ps_sc = ab_ctx.enter_context(tc.tile_pool(name="ps_sc", bufs=2, space="PSUM"))
            ps_pv = ab_ctx.enter_context(tc.tile_pool(name="ps_pv", bufs=2, space="PSUM"))

            # ================= Phase A: QKV projection =================
            # emitted per 512-wide strip; strips 2,3 are interleaved into
            # attention pass 0 (which only needs strips 0,1) as PE filler.
            def emit_a_strip(s4, parts=(0, 1)):
                sl = slice(512 * s4, 512 * (s4 + 1))
                if s4 > 0 and 0 in parts:
                    nc.sync.dma_start(xt[:, :, sl], xt_dr[:, :, sl])
                if 0 not in parts:
                    for_range = ()
                else:
                    for_range = range(4)
                for f in for_range:                         # q0 q1 k0 k1
                    ps = ps_a.tile([128, 512], F32, tag="mm", name=f"qk_{s4}_{f}")
                    for e in range(8):
                        nc.tensor.matmul(
                            ps[:], wqk[:, e, 128 * f:128 * (f + 1)], xt[:, e, sl],
                            start=(e == 0), stop=(e == 7))
                    dst = (qt if f < 2 else kt)[:, f % 2, sl]
                    nc.vector.tensor_tensor(
                        dst, ps[:], bqk[:, f:f + 1].to_broadcast([128, 512]), ALU.add)
                if 1 not in parts:
                    return
                for ss in range(4):                         # V: 128-row blocks
                    s = 4 * s4 + ss
                    ps = ps_a.tile([128, 512], F32, tag="mm", name=f"v_{s4}_{ss}")
                    psv = ps[:, :HG * D]
                    for e in range(8):
                        nc.tensor.matmul(
                            psv, xt[:, e, 128 * s:128 * (s + 1)], wv[:, e, :],
                            start=(e == 0), stop=(e == 7))
                    nc.vector.tensor_tensor(
                        vh[:, s, :, 64:128],
                        psv.rearrange("p (h c) -> p h c", h=HG),
                        bv[:].rearrange("p (h c) -> p h c", h=HG),
                        ALU.add)

            emit_a_strip(0)
            emit_a_strip(1)

            # ================= Phase B: attention, pass-major ================
            # Two passes over the key chunks j per head: pass 0 accumulates
            # query columns [0, 1024) (t=0,1), pass 1 columns [1024, 2048)
            # (t=2,3).  Each (j, column) score is computed exactly once; this
            # halves PV PSUM residency so phase A pools stay open and QKV
            # matmuls fill PE stalls during the exp-bound stretches.  o_proj
            # for each column half is emitted right after the half completes
            # (its PSUM tiles share the "sc" slots), overlapping with the
            # next pass / other heads.
            out_sb = ctx.enter_context(tc.tile_pool(name="out_sb", bufs=8))

            def emit_oproj(half, pool, tag):
                for s in range(8 * half, 8 * (half + 1)):
                    o = out_sb.tile([128, E], BF16, tag="o")
                    for eh in range(2):
                        ps = pool.tile([128, 512], F32, tag=tag,
                                       name=f"oproj_{s}_{eh}")
                        for c in range(2):
                            nc.tensor.matmul(
                                ps[:], outt2[:, c, 128 * s:128 * (s + 1)],
                                wo[:, c, 512 * eh:512 * (eh + 1)],
                                start=(c == 0), stop=(c == 1))
                        if s < 8 or eh == 1:
                            # half-0 copies all on DVE: ACT is the bottleneck
                            # engine during pass 1 when these run
                            nc.vector.tensor_copy(
                                o[:, 512 * eh:512 * (eh + 1)], ps[:])
                        else:
                            nc.scalar.copy(o[:, 512 * eh:512 * (eh + 1)], ps[:])
                    # one DMA per s-chunk (full rows): halves HWDGE issue count
                    nc.sync.dma_start(y_d.ap()[128 * s:128 * (s + 1), :], o[:])

            for P_ in range(2):
                cl, ch = 1024 * P_, 1024 * (P_ + 1)         # column range
                jmax = 8 if P_ == 0 else 16
                for h in range(HG):
                    hk, hp = h // 2, 64 * (h % 2)
                    kts = kt[hp:hp + 64, hk, :]
                    qts = qt[hp:hp + 64, hk, :]
                    pv_tiles = {
                        t: ps_pv.tile([128, 512], F32, tag="pv",
                                      name=f"pv_{h}_{t}")
                        for t in (2 * P_, 2 * P_ + 1)}
                    # the two narrowest j rows of each pass share one psum
                    # tile/exp op (right-aligned, abutting regions): fewer
                    # ACT ops and shorter tail chains per (head, pass) unit
                    if P_ == 0:
                        groups = [(0,), (1,), (2,), (3,), (5, 4), (7, 6)]
                    else:
                        # stop-order constraints: j11 (t=2 stop) after j10,
                        # j15 (t=3 stop) in the last group
                        groups = ([(j,) for j in range(11)]
                                  + [(13, 11), (15, 14, 12)])
                    for grp in groups:
                        # per-member tile column offset (singles: a0-cl;
                        # pairs: packed right-aligned into [B1, 1024))
                        offs = {}
                        pos = 1024
                        for j in sorted(grp, reverse=True):   # wide first
                            a0 = max(128 * j, cl)
                            pos -= ch - a0
                            offs[j] = pos
                        expt = expt_pool.tile([128, 1024], BF16, tag="expt",
                                              name=f"expt_{h}_{P_}_{grp[0]}")
                        sc_ps = ps_sc.tile([128, 1024], F32, tag="sc")
                        for j in grp:
                            lo = 128 * j
                            a0 = max(lo, cl)
                            off = offs[j]
                            for a in range(a0 - a0 % 512, ch, 512):
                                aa = max(a, a0)
                                diag = (aa == lo)       # seg with the diagonal
                                nc.tensor.matmul(
                                    sc_ps[:, off + aa - a0:off + a + 512 - a0],
                                    kts[:, lo:lo + 128], qts[:, aa:a + 512],
                                    start=True, stop=not diag)
                                if diag:
                                    # causal mask: add -1e30 at cols q' < k of
                                    # the 128x128 diagonal tile via I.T @ tneg
                                    nc.tensor.matmul(
                                        sc_ps[:, off:off + 128],
                                        ident[:], tneg[:], start=False,
                                        stop=True)
                        gmin = min(offs.values())
                        nc.scalar.activation(
                            expt[:, gmin:], sc_ps[:, gmin:],
                            AF.Exp, scale=INV_SQRT_D)
                        # PV accumulation (+ denominator row 0 via ones column)
                        for j in sorted(grp):
                            lo = 128 * j
                            a0 = max(lo, cl)
                            off = offs[j]
                            for t in (2 * P_, 2 * P_ + 1):
                                a = max(512 * t, lo)
                                if a >= 512 * (t + 1):
                                    continue
                                nc.tensor.matmul(
                                    pv_tiles[t][:, a - 512 * t:512],
                                    vh[:, j, h, :],
                                    expt[:, off + a - a0:off + 512 * (t + 1) - a0],
                                    start=(j == 0), stop=(j == 4 * t + 3))
                                if j == 4 * t + 3:
                                    # normalize once this sq-chunk completes
                                    rec = dve_tmp.tile([1, 512], F32, tag="rec",
                                                       name=f"rec_{h}_{t}")
                                    bc = dve_tmp.tile([128, 512], F32, tag="bc",
                                                      name=f"bc_{h}_{t}")
                                    nc.vector.reciprocal(rec[:],
                                                         pv_tiles[t][0:1, :])
                                    nc.gpsimd.partition_broadcast(bc[:], rec[:])
                                    tsl = slice(512 * t, 512 * (t + 1))
                                    if h % 2 == 1:
                                        # odd heads: partitions already match
                                        # outt2's upper half — write direct
                                        nc.vector.tensor_tensor(
                                            outt2[64:128, h // 2, tsl],
                                            pv_tiles[t][64:128, :],
                                            bc[64:128, :], ALU.mult)
                                    else:
                                        nc.vector.tensor_tensor(
                                            outt[64:128, h // 2, tsl],
                                            pv_tiles[t][64:128, :],
                                            bc[64:128, :], ALU.mult)
                                        # cross-partition stack to rows 0-63
                                        nc.sync.dma_start(
                                            outt2[0:64, h // 2, tsl],
                                            outt[64:128, h // 2, tsl])

                    if P_ == 0:
                        # PE filler during the exp-bound stretches
                        [lambda: emit_a_strip(2, (0,)),
                         lambda: emit_a_strip(2, (1,)),
                         lambda: emit_a_strip(3, (0,)),
                         lambda: emit_a_strip(3, (1,))][h]()
                    if P_ == 1 and h == 1:
                        emit_oproj(0, ps_a, "mm")           # low-priority filler

            # o_proj half 1 runs after the A/B pools close, with a deeper
            # dedicated PSUM pool for a tighter copy/DMA pipeline.
            ab_ctx.close()
            with tc.tile_pool(name="ps_c", bufs=6, space="PSUM") as ps_c:
                emit_oproj(1, ps_c, "oproj")
    nc.compile()
    return nc


_NC_CACHE = {}


def _get_nc():
    if "nc" not in _NC_CACHE:
        _NC_CACHE["nc"] = build_nc()
    return _NC_CACHE["nc"]


def kernel(X, mask, W_qkv, b_qkv, W_o, b_o):
    X = np.asarray(X, dtype=np.float32)
    W_qkv = np.asarray(W_qkv, dtype=np.float32)
    b_qkv = np.asarray(b_qkv, dtype=np.float32)
    W_o = np.asarray(W_o, dtype=np.float32)
    b_o = np.asarray(b_o, dtype=np.float32)

    ident = np.eye(128, dtype=np.float32).astype(BF)
    r = np.arange(128)
    tneg = np.where(r[None, :] < r[:, None], np.float32(-1e30), np.float32(0.0))
    tneg = tneg.astype(BF)                      # tneg[p, n] = -1e30 if n < p

    in_maps = []
    for c in range(NC_):
        b, g = c // 4, c % 4
        cols = slice(256 * g, 256 * (g + 1))
        xt = np.ascontiguousarray(X[b].T).astype(BF)
        wqk = np.concatenate(
            [W_qkv[:, cols], W_qkv[:, 1024 + 256 * g:1024 + 256 * (g + 1)]],
            axis=1).astype(BF)
        wv = np.ascontiguousarray(W_qkv[:, 2048 + 256 * g:2048 + 256 * (g + 1)]).astype(BF)
        wo = np.ascontiguousarray(
            W_o[256 * g:256 * (g + 1), :].reshape(2, 128, E).transpose(1, 0, 2)).astype(BF)
        bqk = np.concatenate(
            [b_qkv[cols], b_qkv[1024 + 256 * g:1024 + 256 * (g + 1)]]
        ).reshape(4, 128).T.copy().astype(np.float32)
        bv = np.broadcast_to(
            b_qkv[2048 + 256 * g:2048 + 256 * (g + 1)], (128, 256)).copy().astype(np.float32)
        in_maps.append({"xt": xt, "wqk": wqk, "wv": wv, "wo": wo,
                        "ident": ident, "tneg": tneg, "bqk": bqk, "bv": bv})

    nc = _get_nc()
    res = run_bass_kernel_spmd(nc, in_maps, core_ids=list(range(NC_)))

    Y = np.zeros((B, S, E), dtype=np.float32)
    for c in range(NC_):
        Y[c // 4] += res.results[c]["y"].astype(np.float32)
    Y += b_o[None, None, :]
    return Y



# revision 11
# speedup vs baseline: 1.0121x; 1.0121x over previous
"""Trainium2 Bass kernel for nn_MultiHeadAttention (B=2, S=2048, E=1024, H=16, d=64).

Sharding: 8 cores = 2 batches x 4 head-groups (4 heads each).
Per core: QKV projection (transposed layout), causal flash-style attention
(transposed softmax, no max subtraction), o_proj partial; host sums the
4 partials per batch (the tensor-parallel all-reduce, done at gather time).

All matmuls in bf16 with fp32 PSUM accumulation.
"""
import sys

sys.path.insert(0, "/opt/trn_rl_repo")

import math

import ml_dtypes
import numpy as np

import concourse.bacc as bacc_mod
import concourse.tile as tile
from concourse import mybir
from concourse.bass_utils import run_bass_kernel_spmd

F32 = mybir.dt.float32
BF16 = mybir.dt.bfloat16
FP8 = mybir.dt.float8e4
AF = mybir.ActivationFunctionType
ALU = mybir.AluOpType
DR = mybir.MatmulPerfMode.DoubleRow

B, S, E = 2, 2048, 1024
H, D = 16, 64           # total heads, head dim
HG = 4                  # heads per core (group)
NC_ = 8                 # cores
SC = S // 128           # 16 sequence chunks of 128
INV_SQRT_D = 1.0 / math.sqrt(D)

BF = ml_dtypes.bfloat16
F8 = ml_dtypes.float8_e4m3


def build_nc():
    nc = bacc_mod.Bacc(target_bir_lowering=False)

    # ---- DRAM tensors (per-core shards, prepared on host) ----
    # fp8 DoubleRow compensated QKV: qkv = X8@W8 + Xr8@W8 + (X8/16)@(16*Wr)
    xt8_d = nc.dram_tensor("xt8", [E, S], FP8, kind="ExternalInput")   # q8(X[b].T)
    xtr_d = nc.dram_tensor("xtr", [E, S], FP8, kind="ExternalInput")   # q8(X - X8)
    xtb_d = nc.dram_tensor("xtb", [E, S], FP8, kind="ExternalInput")   # X8/16
    wqk8_d = nc.dram_tensor("wqk8", [E, 2 * HG * D], FP8, kind="ExternalInput")
    wqkr_d = nc.dram_tensor("wqkr", [E, 2 * HG * D], FP8, kind="ExternalInput")
    wv8_d = nc.dram_tensor("wv8", [E, HG * D], FP8, kind="ExternalInput")
    wvr_d = nc.dram_tensor("wvr", [E, HG * D], FP8, kind="ExternalInput")
    wo_d = nc.dram_tensor("wo", [128, 2, E], BF16, kind="ExternalInput")
    ident_d = nc.dram_tensor("ident", [128, 128], BF16, kind="ExternalInput")
    tneg_d = nc.dram_tensor("tneg", [128, 128], BF16, kind="ExternalInput")
    bqk_d = nc.dram_tensor("bqk", [128, 4], F32, kind="ExternalInput")
    bv_d = nc.dram_tensor("bv", [128, HG * D], F32, kind="ExternalInput")
    y_d = nc.dram_tensor("y", [S, E], BF16, kind="ExternalOutput")

    with tile.TileContext(nc) as tc:
        import contextlib
        with contextlib.ExitStack() as ctx:
            persist = ctx.enter_context(tc.tile_pool(name="persist", bufs=1))
            dve_tmp = ctx.enter_context(tc.tile_pool(name="dve_tmp", bufs=4))
            expt_pool = ctx.enter_context(tc.tile_pool(name="expt_pool", bufs=7))

            # ---- persistent SBUF tiles ----
            xt8 = persist.tile([128, 8, S], FP8)            # X.T  [e-part, e-chunk, s]
            xtr = persist.tile([128, 8, S], FP8)
            xtb = persist.tile([128, 8, S], FP8)
            wqk8 = persist.tile([128, 8, 2 * HG * D], FP8)
            wqkr = persist.tile([128, 8, 2 * HG * D], FP8)
            wv8 = persist.tile([128, 8, HG * D], FP8)
            wvr = persist.tile([128, 8, HG * D], FP8)
            wo = persist.tile([128, 2, E], BF16)
            ident = persist.tile([128, 128], BF16)
            tneg = persist.tile([128, 128], BF16)
            bqk = persist.tile([128, 4], F32)
            bv = persist.tile([128, HG * D], F32)
            qt = persist.tile([128, 2, S], BF16)            # Q.T
            kt = persist.tile([128, 2, S], BF16)            # K.T
            # V-hat: col 0 = ones (denominator row), cols 1-63 zero pad
            # (engine partition APs must fit aligned power-of-2 blocks),
            # cols 64-127 = V
            vh = persist.tile([128, SC, HG, 128], BF16)
            outt = persist.tile([128, 2, S], BF16)          # even-head staging (rows 64-127)
            outt2 = persist.tile([128, 2, S], BF16)         # stacked for o_proj

            _r = lambda d: d.ap().rearrange("(o p) s -> p o s", p=128)
            xt8_dr, xtr_dr, xtb_dr = _r(xt8_d), _r(xtr_d), _r(xtb_d)
            # strip-0 loads in the order phase A consumes them: main term
            # (wqk8 + xt8), then X-residual (xtr), then W-residual (xtb+wqkr)
            nc.sync.dma_start(wqk8[:], _r(wqk8_d))
            nc.sync.dma_start(xt8[:, :, 0:512], xt8_dr[:, :, 0:512])
            nc.sync.dma_start(xtr[:, :, 0:512], xtr_dr[:, :, 0:512])
            nc.sync.dma_start(wqkr[:], _r(wqkr_d))
            nc.sync.dma_start(xtb[:, :, 0:512], xtb_dr[:, :, 0:512])
            nc.sync.dma_start(wv8[:], _r(wv8_d))
            nc.sync.dma_start(wvr[:], _r(wvr_d))
            nc.sync.dma_start(ident[:], ident_d.ap())
            nc.sync.dma_start(tneg[:], tneg_d.ap())
            nc.sync.dma_start(bqk[:], bqk_d.ap())
            nc.sync.dma_start(bv[:], bv_d.ap())
            nc.sync.dma_start(wo[:], wo_d.ap())
            # vh cols 1-63 stay uninitialized: they only feed PSUM rows 1-63,
            # which are never read (row 0 = denom, rows 64-127 = V out)
            nc.vector.memset(vh[:, :, :, 0], 1.0)

            ab_ctx = ctx.enter_context(contextlib.ExitStack())
            ps_a = ab_ctx.enter_context(tc.tile_pool(name="ps_a", bufs=2, space="PSUM"))
            ps_sc = ab_ctx.enter_context(tc.tile_pool(name="ps_sc", bufs=2, space="PSUM"))
            ps_pv = ab_ctx.enter_context(tc.tile_pool(name="ps_pv", bufs=2, space="PSUM"))

            # ================= Phase A: QKV projection =================
            # emitted per 512-wide strip; strips 2,3 are interleaved into
            # attention pass 0 (which only needs strips 0,1) as PE filler.
            # compensated-fp8 accumulation: 3 terms x 4 e-pairs, DoubleRow
            # (2 k-tiles per call at 0.5 cyc/col -> 25% fewer PE cycles than
            # bf16 with full-precision-class accuracy)
            A_TERMS = [(wqk8, xt8), (wqk8, xtr), (wqkr, xtb)]

            def emit_a_strip(s4, parts=(0, 1)):
                sl = slice(512 * s4, 512 * (s4 + 1))
                if s4 > 0 and 0 in parts:
                    nc.sync.dma_start(xt8[:, :, sl], xt8_dr[:, :, sl])
                    nc.sync.dma_start(xtr[:, :, sl], xtr_dr[:, :, sl])
                    nc.sync.dma_start(xtb[:, :, sl], xtb_dr[:, :, sl])
                if 0 not in parts:
                    for_range = ()
                else:
                    for_range = range(4)
                for f in for_range:                         # q0 q1 k0 k1
                    ps = ps_a.tile([128, 512], F32, tag="mm", name=f"qk_{s4}_{f}")
                    for i, (w_, x_) in enumerate(A_TERMS):
                        for e2 in range(4):
                            nc.tensor.matmul(
                                ps[:], w_[:, 2 * e2:2 * e2 + 2, 128 * f:128 * (f + 1)],
                                x_[:, 2 * e2:2 * e2 + 2, sl],
                                start=(i == 0 and e2 == 0),
                                stop=(i == 2 and e2 == 3), perf_mode=DR)
                    dst = (qt if f < 2 else kt)[:, f % 2, sl]
                    nc.vector.tensor_tensor(
                        dst, ps[:], bqk[:, f:f + 1].to_broadcast([128, 512]), ALU.add)
                if 1 not in parts:
                    return
                for ss in range(4):                         # V: 128-row blocks
                    s = 4 * s4 + ss
                    ps = ps_a.tile([128, 512], F32, tag="mm", name=f"v_{s4}_{ss}")
                    psv = ps[:, :HG * D]
                    for i, (w_, x_) in enumerate([(wv8, xt8), (wv8, xtr), (wvr, xtb)]):
                        for e2 in range(4):
                            nc.tensor.matmul(
                                psv, x_[:, 2 * e2:2 * e2 + 2, 128 * s:128 * (s + 1)],
                                w_[:, 2 * e2:2 * e2 + 2, :],
                                start=(i == 0 and e2 == 0),
                                stop=(i == 2 and e2 == 3), perf_mode=DR)
                    nc.vector.tensor_tensor(
                        vh[:, s, :, 64:128],
                        psv.rearrange("p (h c) -> p h c", h=HG),
                        bv[:].rearrange("p (h c) -> p h c", h=HG),
                        ALU.add)

            emit_a_strip(0)
            emit_a_strip(1)

            # ================= Phase B: attention, pass-major ================
            # Two passes over the key chunks j per head: pass 0 accumulates
            # query columns [0, 1024) (t=0,1), pass 1 columns [1024, 2048)
            # (t=2,3).  Each (j, column) score is computed exactly once; this
            # halves PV PSUM residency so phase A pools stay open and QKV
            # matmuls fill PE stalls during the exp-bound stretches.  o_proj
            # for each column half is emitted right after the half completes
            # (its PSUM tiles share the "sc" slots), overlapping with the
            # next pass / other heads.
            out_sb = ctx.enter_context(tc.tile_pool(name="out_sb", bufs=8))

            def emit_oproj(s_list, pool, tag, eng=("dve", "dve")):
                for s in s_list:
                    o = out_sb.tile([128, E], BF16, tag="o")
                    for eh in range(2):
                        ps = pool.tile([128, 512], F32, tag=tag,
                                       name=f"oproj_{s}_{eh}")
                        for c in range(2):
                            nc.tensor.matmul(
                                ps[:], outt2[:, c, 128 * s:128 * (s + 1)],
                                wo[:, c, 512 * eh:512 * (eh + 1)],
                                start=(c == 0), stop=(c == 1))
                        if eng[eh] == "dve":
                            nc.vector.tensor_copy(
                                o[:, 512 * eh:512 * (eh + 1)], ps[:])
                        else:
                            nc.scalar.copy(o[:, 512 * eh:512 * (eh + 1)], ps[:])
                    # one DMA per s-chunk (full rows): halves HWDGE issue count
                    nc.sync.dma_start(y_d.ap()[128 * s:128 * (s + 1), :], o[:])

            for P_ in range(2):
                cl, ch = 1024 * P_, 1024 * (P_ + 1)         # column range
                jmax = 8 if P_ == 0 else 16
                for h in range(HG):
                    hk, hp = h // 2, 64 * (h % 2)
                    kts = kt[hp:hp + 64, hk, :]
                    qts = qt[hp:hp + 64, hk, :]
                    pv_tiles = {
                        t: ps_pv.tile([128, 512], F32, tag="pv",
                                      name=f"pv_{h}_{t}")
                        for t in (2 * P_, 2 * P_ + 1)}
                    # the two narrowest j rows of each pass share one psum
                    # tile/exp op (right-aligned, abutting regions): fewer
                    # ACT ops and shorter tail chains per (head, pass) unit
                    if P_ == 0:
                        groups = [(0,), (1,), (2,), (3,), (5, 4), (7, 6)]
                    else:
                        # stop-order constraints: j11 (t=2 stop) after j10,
                        # j15 (t=3 stop) in the last group
                        groups = ([(j,) for j in range(11)]
                                  + [(13, 11), (15, 14, 12)])
                    for grp in groups:
                        # per-member tile column offset (singles: a0-cl;
                        # pairs: packed right-aligned into [B1, 1024))
                        offs = {}
                        pos = 1024
                        for j in sorted(grp, reverse=True):   # wide first
                            a0 = max(128 * j, cl)
                            pos -= ch - a0
                            offs[j] = pos
                        expt = expt_pool.tile([128, 1024], BF16, tag="expt",
                                              name=f"expt_{h}_{P_}_{grp[0]}")
                        sc_ps = ps_sc.tile([128, 1024], F32, tag="sc")
                        for j in grp:
                            lo = 128 * j
                            a0 = max(lo, cl)
                            off = offs[j]
                            for a in range(a0 - a0 % 512, ch, 512):
                                aa = max(a, a0)
                                diag = (aa == lo)       # seg with the diagonal
                                nc.tensor.matmul(
                                    sc_ps[:, off + aa - a0:off + a + 512 - a0],
                                    kts[:, lo:lo + 128], qts[:, aa:a + 512],
                                    start=True, stop=not diag)
                                if diag:
                                    # causal mask: add -1e30 at cols q' < k of
                                    # the 128x128 diagonal tile via I.T @ tneg
                                    nc.tensor.matmul(
                                        sc_ps[:, off:off + 128],
                                        ident[:], tneg[:], start=False,
                                        stop=True)
                        gmin = min(offs.values())
                        nc.scalar.activation(
                            expt[:, gmin:], sc_ps[:, gmin:],
                            AF.Exp, scale=INV_SQRT_D)
                        # PV accumulation (+ denominator row 0 via ones column)
                        for j in sorted(grp):
                            lo = 128 * j
                            a0 = max(lo, cl)
                            off = offs[j]
                            for t in (2 * P_, 2 * P_ + 1):
                                a = max(512 * t, lo)
                                if a >= 512 * (t + 1):
                                    continue
                                nc.tensor.matmul(
                                    pv_tiles[t][:, a - 512 * t:512],
                                    vh[:, j, h, :],
                                    expt[:, off + a - a0:off + 512 * (t + 1) - a0],
                                    start=(j == 0), stop=(j == 4 * t + 3))
                                if j == 4 * t + 3:
                                    # normalize once this sq-chunk completes
                                    rec = dve_tmp.tile([1, 512], F32, tag="rec",
                                                       name=f"rec_{h}_{t}")
                                    bc = dve_tmp.tile([128, 512], F32, tag="bc",
                                                      name=f"bc_{h}_{t}")
                                    nc.vector.reciprocal(rec[:],
                                                         pv_tiles[t][0:1, :])
                                    nc.gpsimd.partition_broadcast(bc[:], rec[:])
                                    tsl = slice(512 * t, 512 * (t + 1))
                                    if h % 2 == 1:
                                        # odd heads: partitions already match
                                        # outt2's upper half — write direct
                                        nc.vector.tensor_tensor(
                                            outt2[64:128, h // 2, tsl],
                                            pv_tiles[t][64:128, :],
                                            bc[64:128, :], ALU.mult)
                                    else:
                                        nc.vector.tensor_tensor(
                                            outt[64:128, h // 2, tsl],
                                            pv_tiles[t][64:128, :],
                                            bc[64:128, :], ALU.mult)
                                        # cross-partition stack to rows 0-63
                                        nc.sync.dma_start(
                                            outt2[0:64, h // 2, tsl],
                                            outt[64:128, h // 2, tsl])
                        if P_ == 1 and h == 3 and 11 in grp:
                            # all heads' t=2 columns are final: overlap their
                            # o_proj with the last head's t=3 tail
                            emit_oproj(range(8, 12), ps_a, "mm")

                    if P_ == 0:
                        # PE filler during the exp-bound stretches
                        [lambda: emit_a_strip(2, (0,)),
                         lambda: emit_a_strip(2, (1,)),
                         lambda: emit_a_strip(3, (0,)),
                         lambda: emit_a_strip(3, (1,))][h]()
                    if P_ == 1 and h == 1:
                        emit_oproj(range(0, 8), ps_a, "mm")  # low-priority filler

            # s=12..15 run after the A/B pools close, with a deeper dedicated
            # PSUM pool for a tighter copy/DMA pipeline (s=8..11 were
            # interleaved into head 3's pass-1 stretch above).
            ab_ctx.close()
            with tc.tile_pool(name="ps_c", bufs=6, space="PSUM") as ps_c:
                emit_oproj(range(12, 16), ps_c, "oproj", eng=("act", "dve"))
    nc.compile()
    return nc


_NC_CACHE = {}


def _get_nc():
    if "nc" not in _NC_CACHE:
        _NC_CACHE["nc"] = build_nc()
    return _NC_CACHE["nc"]


def kernel(X, mask, W_qkv, b_qkv, W_o, b_o):
    X = np.asarray(X, dtype=np.float32)
    W_qkv = np.asarray(W_qkv, dtype=np.float32)
    b_qkv = np.asarray(b_qkv, dtype=np.float32)
    W_o = np.asarray(W_o, dtype=np.float32)
    b_o = np.asarray(b_o, dtype=np.float32)

    ident = np.eye(128, dtype=np.float32).astype(BF)
    r = np.arange(128)
    tneg = np.where(r[None, :] < r[:, None], np.float32(-1e30), np.float32(0.0))
    tneg = tneg.astype(BF)                      # tneg[p, n] = -1e30 if n < p

    # fp8 compensation operands (dtype prep for the device kernel):
    # X ~ X8 + Xr8;  W ~ W8 + Wr16/16;  X8/16 is an exact exponent shift
    xv = []
    for b in range(B):
        xT = np.ascontiguousarray(X[b].T)
        x8 = xT.astype(F8)
        xr = (xT - x8.astype(np.float32)).astype(F8)
        xb = (x8.astype(np.float32) / 16.0).astype(F8)
        xv.append((x8, xr, xb))
    W8f = W_qkv.astype(F8).astype(np.float32)
    Wr16 = (16.0 * (W_qkv - W8f)).astype(F8)
    W8 = W8f.astype(F8)

    in_maps = []
    for c in range(NC_):
        b, g = c // 4, c % 4
        cols = slice(256 * g, 256 * (g + 1))
        kcols = slice(1024 + 256 * g, 1024 + 256 * (g + 1))
        vcols = slice(2048 + 256 * g, 2048 + 256 * (g + 1))
        wqk8 = np.concatenate([W8[:, cols], W8[:, kcols]], axis=1)
        wqkr = np.concatenate([Wr16[:, cols], Wr16[:, kcols]], axis=1)
        wv8 = np.ascontiguousarray(W8[:, vcols])
        wvr = np.ascontiguousarray(Wr16[:, vcols])
        wo = np.ascontiguousarray(
            W_o[256 * g:256 * (g + 1), :].reshape(2, 128, E).transpose(1, 0, 2)).astype(BF)
        bqk = np.concatenate(
            [b_qkv[cols], b_qkv[kcols]]
        ).reshape(4, 128).T.copy().astype(np.float32)
        bv = np.broadcast_to(
            b_qkv[vcols], (128, 256)).copy().astype(np.float32)
        in_maps.append({"xt8": xv[b][0], "xtr": xv[b][1], "xtb": xv[b][2],
                        "wqk8": wqk8, "wqkr": wqkr, "wv8": wv8, "wvr": wvr,
                        "wo": wo, "ident": ident, "tneg": tneg,
                        "bqk": bqk, "bv": bv})

    nc = _get_nc()
    res = run_bass_kernel_spmd(nc, in_maps, core_ids=list(range(NC_)))

    Y = np.zeros((B, S, E), dtype=np.float32)
    for c in range(NC_):
        Y[c // 4] += res.results[c]["y"].astype(np.float32)
    Y += b_o[None, None, :]
    return Y

